# revision 21
# baseline (speedup 1.0000x reference)
"""Trainium2 Bass kernel for a 2-layer LSTM decoder (nn_Decoder_recurrent).

Strategy (8 NeuronCores, data-parallel over batch):
  - Each core handles B_local = 1024/8 = 128 batch rows for all T=336 steps.
  - Weights resident in SBUF; W_hh0.T and W_hh1.T as scaled fp8e4 run in
    DoubleRow perf mode (2 fp8 MACs/PE cell/cycle, halves those matmuls);
    W_ih1.T stays fp16 for accuracy (its quantization noise dominated the
    fp8 error budget), pre-scaled by 2^16 so every contribution to a gate
    PSUM group shares one scale that the ScalarE activation unwinds for
    free via its `scale` operand.
  - Scales: fp8 weights x1024, fp8 hidden-state stationaries x64.
  - Gate matmuls run activations-stationary: lhsT = h^T tiles, rhs = W.T
    streamed 512 columns per PSUM bank group.
  - The small input matmuls (y_prev/features/biases via k9pack, layer-1
    bias via xbias) stay fp16; output projection y = W_out h1 + b is fp16.
  - Latency hiding: layer-1's W_hh1 (A) matmuls are emitted ahead of the
    layer-0 transpose; the NEXT step's layer-0 DR matmuls are emitted ahead
    of the layer-1 transpose; cell math is half-chunked (fp16 temps) so
    PE transposes and the ScalarE tanh pipeline with the VectorE chain.
"""

import sys

sys.path.insert(0, '/opt/trn_rl_repo')

import numpy as np
import ml_dtypes

import concourse.bass as bass
import concourse.tile as tile
from concourse import mybir
import bass_rust
from concourse.bass_utils import run_bass_kernel_spmd
from concourse.masks import make_identity

B, T, M, Q = 1024, 336, 1, 9
DFF, H, L = 7, 1024, 2
NCORES = 8
BL = B // NCORES        # 128 batch rows per core
KT = H // 128           # 8 K-tiles per hidden matmul
G4H = 4 * H             # 4096 gate columns
NG = 512                # gate psum group width (one PSUM bank of fp32)
NGROUPS = G4H // NG     # 8
HH = H // 2             # cell-math half-chunk

WS = 1024.0             # fp8 weight scale
HS = 64.0               # fp8 hidden-state scale
GS = WS * HS            # PSUM gate scale (2^16)

f8 = mybir.dt.float8e4
f16 = mybir.dt.float16
f32 = mybir.dt.float32

SIG = mybir.ActivationFunctionType.Sigmoid
TANH = mybir.ActivationFunctionType.Tanh
IDENT = mybir.ActivationFunctionType.Identity
DR = mybir.MatmulPerfMode.DoubleRow

_module_cache = {}


def _split_multi_waits(nc, max_waits=1):
    """This container's walrus accepts at most one sem-wait per instruction;
    hoist extras onto same-engine NoOps placed immediately before."""
    for f in nc.m.functions:
        for bb in f.blocks:
            new_insts = []
            for inst in bb.instructions:
                si = inst.sync_info
                if si is not None and si.on_wait and len(si.on_wait) > max_waits:
                    waits = list(si.on_wait)
                    for j, w in enumerate(waits[max_waits:]):
                        nop = bass_rust.InstNoOp(
                            name=f"{inst.name}-sw{j}", ins=[], outs=[])
                        nop.engine = inst.engine
                        nop.sync_info = mybir.SyncInfo(on_wait=[w], on_update=[])
                        new_insts.append(nop)
                    si.on_wait = waits[:max_waits]
                new_insts.append(inst)
            bb.instructions = new_insts


def _build_module(Tsteps):
    nc = bass.Bass("TRN2", target_bir_lowering=False)

    d_whh0 = nc.dram_tensor("whh0t", [128, KT, G4H], f8, kind="ExternalInput")
    d_whh1 = nc.dram_tensor("whh1t", [128, KT, G4H], f8, kind="ExternalInput")
    d_wih1 = nc.dram_tensor("wih1t", [128, KT, G4H], f8, kind="ExternalInput")
    # fp16 copy of W_ih1.T's g-gate (cell candidate) columns: tanh(g) noise
    # feeds the cell state with gain ~1, so those columns stay fp16
    d_wih1g = nc.dram_tensor("wih1g", [128, KT, 2 * NG], f16,
                             kind="ExternalInput")
    # k9pack rows (x 2^16): 0 = W_ih0[:,0] (y weight), 1-7 = W_ih0[:,1:8].T,
    # 8 = b_ih0+b_hh0, 9 = b_ih1+b_hh1, 10-127 = zeros
    d_k9 = nc.dram_tensor("k9pack", [128, G4H], f16, kind="ExternalInput")
    d_wout = nc.dram_tensor("woutt", [128, KT, Q], f16, kind="ExternalInput")
    d_bout = nc.dram_tensor("bout", [Q, 1], f32, kind="ExternalInput")
    d_xbias = nc.dram_tensor("xbias", [128, 128], f16, kind="ExternalInput")
    d_h0t8 = nc.dram_tensor("h0t8", [128, KT, BL], f8, kind="ExternalInput")
    d_h1t8 = nc.dram_tensor("h1t8", [128, KT, BL], f8, kind="ExternalInput")
    d_h0t16 = nc.dram_tensor("h0t16", [128, KT, BL], f16, kind="ExternalInput")
    d_h1t16 = nc.dram_tensor("h1t16", [128, KT, BL], f16, kind="ExternalInput")
    d_c = nc.dram_tensor("cinit", [L, BL, H], f32, kind="ExternalInput")
    # ffy rows: 0 = y_prev slot (host fills t=0 only), 1-7 = f_t, 8 = ones
    d_ffy = nc.dram_tensor("ffy", [Tsteps, 9, BL], f16, kind="ExternalInput")
    d_y = nc.dram_tensor("yout", [Tsteps, Q, BL], f32, kind="ExternalOutput")

    with tile.TileContext(nc) as tc:
        with (
            tc.tile_pool(name="wres", bufs=1) as wres,
            tc.tile_pool(name="state", bufs=1) as state,
            tc.tile_pool(name="acttmp", bufs=1) as acttmp,
            tc.tile_pool(name="dvetmp", bufs=1) as dvetmp,
            tc.tile_pool(name="ytp", bufs=3) as ytp,
            tc.tile_pool(name="gpsum", bufs=5, space="PSUM") as gpsum,
            tc.tile_pool(name="tpsum", bufs=2, space="PSUM") as tpsum,
            tc.tile_pool(name="ypsum", bufs=1, space="PSUM") as ypsum,
        ):
            w_hh0 = wres.tile([128, KT, G4H], f8, tag="w_hh0")
            w_hh1 = wres.tile([128, KT, G4H], f8, tag="w_hh1")
            w_ih1 = wres.tile([128, KT, G4H], f8, tag="w_ih1")
            w_ih1g = wres.tile([128, KT, 2 * NG], f16, tag="w_ih1g")
            k9 = wres.tile([128, G4H], f16, tag="k9")
            w_out = wres.tile([128, KT, Q], f16, tag="w_out")
            b_out = wres.tile([Q, 1], f32, tag="b_out")
            ident = wres.tile([128, 128], f16, tag="ident")
            nc.sync.dma_start(w_hh0[:], d_whh0[:])
            nc.sync.dma_start(w_hh1[:], d_whh1[:])
            nc.sync.dma_start(w_ih1[:], d_wih1[:])
            nc.sync.dma_start(w_ih1g[:], d_wih1g[:])
            nc.sync.dma_start(k9[:], d_k9[:])
            nc.sync.dma_start(w_out[:], d_wout[:])
            nc.sync.dma_start(b_out[:], d_bout[:])
            make_identity(nc, ident[:])

            h0T8 = state.tile([128, KT, BL], f8, tag="h0T8")
            h1T8 = state.tile([128, KT, BL], f8, tag="h1T8")
            h0T16 = state.tile([128, KT, BL], f16, tag="h0T16")
            h1T16 = state.tile([128, KT, BL], f16, tag="h1T16")
            c0 = state.tile([BL, H], f32, tag="c0")
            c1 = state.tile([BL, H], f32, tag="c1")
            nc.sync.dma_start(h0T8[:], d_h0t8[:])
            nc.sync.dma_start(h1T8[:], d_h1t8[:])
            nc.sync.dma_start(h0T16[:], d_h0t16[:])
            nc.sync.dma_start(h1T16[:], d_h1t16[:])
            nc.sync.dma_start(c0[:], d_c[0])
            nc.sync.dma_start(c1[:], d_c[1])

            # bias-only stationary for layer 1: row 9 = ones, rest zeros
            xbias = state.tile([128, 128], f16, tag="xbias")
            nc.sync.dma_start(xbias[:], d_xbias[:])

            # rotating per-step input stationaries (rows 9-127 stay zero)
            xaug = [state.tile([128, BL], f16, tag=f"xaug{i}", name=f"xaug{i}")
                    for i in range(3)]
            for xt in xaug:
                nc.vector.memset(xt[:], 0.0)

            # fp16 activation temps (full gate blocks) and fp16 cell temp
            si = [acttmp.tile([BL, H], f16, tag=f"si{l}", name=f"si{l}")
                  for l in range(2)]
            sf = [acttmp.tile([BL, H], f16, tag=f"sf{l}", name=f"sf{l}")
                  for l in range(2)]
            tg = [acttmp.tile([BL, H], f16, tag=f"tg{l}", name=f"tg{l}")
                  for l in range(2)]
            so = [acttmp.tile([BL, H], f16, tag=f"so{l}", name=f"so{l}")
                  for l in range(2)]
            tc_ = [acttmp.tile([BL, H], f16, tag=f"tc{l}", name=f"tc{l}")
                   for l in range(2)]
            hn = [acttmp.tile([BL, H], f16, tag=f"hn{l}", name=f"hn{l}")
                  for l in range(2)]
            t1 = dvetmp.tile([BL, H], f16, tag="t1")

            cs = [c0, c1]

            def dr_mms(ps, hT8, w8, g, start):
                """4 DoubleRow MMs accumulating one 512-col gate group."""
                for kv in range(KT // 2):
                    nc.tensor.matmul(
                        ps[:], hT8[:, 2 * kv:2 * kv + 2, :],
                        w8[:, 2 * kv:2 * kv + 2, g * NG:(g + 1) * NG],
                        start=(start and kv == 0), stop=False,
                        perf_mode=DR)

            def fp16_mms(ps, hT16, w16, col0, start):
                """8 fp16 MMs accumulating one 512-col gate group."""
                for k in range(KT):
                    nc.tensor.matmul(
                        ps[:], hT16[:, k, :],
                        w16[:, k, col0:col0 + NG],
                        start=(start and k == 0), stop=False)

            def act_for_group(layer, g, ps):
                """ACT nonlinearity for gate psum group g -> fp16 SBUF."""
                blk, half = divmod(g, 2)
                dst = (si, sf, tg, so)[blk][layer]
                func = TANH if blk == 2 else SIG
                nc.scalar.activation(
                    dst[:, half * NG:(half + 1) * NG], ps[:], func,
                    scale=1.0 / GS)

            def cell_math(layer):
                """c = sig(f)*c + sig(i)*tanh(g); h = sig(o)*tanh(c).
                Half-chunked so ScalarE tanh overlaps the VectorE chain."""
                c = cs[layer]
                for hh in range(2):
                    s = slice(hh * HH, (hh + 1) * HH)
                    nc.vector.tensor_tensor(t1[:, s], si[layer][:, s],
                                            tg[layer][:, s],
                                            mybir.AluOpType.mult)
                    nc.vector.tensor_tensor(c[:, s], c[:, s], sf[layer][:, s],
                                            mybir.AluOpType.mult)
                    nc.vector.tensor_tensor(c[:, s], c[:, s], t1[:, s],
                                            mybir.AluOpType.add)
                    nc.scalar.activation(tc_[layer][:, s], c[:, s], TANH)
                for hh in range(2):
                    s = slice(hh * HH, (hh + 1) * HH)
                    nc.vector.tensor_tensor(hn[layer][:, s], so[layer][:, s],
                                            tc_[layer][:, s],
                                            mybir.AluOpType.mult)

            def transpose_h(layer):
                """PE-transpose hn -> hT tiles. Each half gets its own PSUM
                tile (separate banks) so the second half's transposes (PE
                writes) can overlap the first half's copies (DVE/ACT reads);
                same-bank PE-write + engine-read is a fatal hazard the
                framework would otherwise serialize around.
                Layer 0: B's DR matmuls need the fp8 copy first (DVE), the
                fp16 g-gate copy rides on ACT. Layer 1: y-proj needs the
                fp16 copy first; the fp8 one (next step's A) follows on DVE."""
                hT16 = (h0T16, h1T16)[layer]
                hT8 = (h0T8, h1T8)[layer]
                tps = []
                for half in range(2):
                    tph = tpsum.tile([128, 4, BL], f16, tag="tp")
                    tps.append(tph)
                    for jj in range(4):
                        j = 4 * half + jj
                        nc.tensor.transpose(tph[:, jj, :],
                                            hn[layer][:, j * 128:(j + 1) * 128],
                                            ident[:])
                    hs = slice(4 * half, 4 * half + 4)
                    if layer == 0:
                        nc.vector.tensor_scalar_mul(hT8[:, hs, :], tph[:], HS)
                        nc.scalar.activation(hT16[:, hs, :], tph[:], IDENT)
                    else:
                        nc.vector.tensor_copy(hT16[:, hs, :], tph[:])
                if layer == 1:
                    for half in range(2):
                        hs = slice(4 * half, 4 * half + 4)
                        nc.vector.tensor_scalar_mul(hT8[:, hs, :],
                                                    tps[half][:], HS)

            # gate-group emission order: cell math consumes i (g0-1) and
            # tanh-g (g4-5) first, then f (g2-3), with o (g6-7) last - so
            # emitting groups in that order lets the VectorE cell chain
            # overlap the gate matmul stream instead of trailing it.
            PRE_GROUPS = (0, 4, 1, 5)
            FULL_GROUPS = (2, 3, 6, 7)
            ps_l0 = [None] * NGROUPS

            for t in range(Tsteps):
                xa = xaug[t % 3]
                if t == 0:
                    nc.sync.dma_start(xa[0:9, :], d_ffy[t, 0:9, :])
                    for g in PRE_GROUPS:
                        ps = gpsum.tile([BL, NG], f32, tag="gps")
                        ps_l0[g] = ps
                        dr_mms(ps, h0T8, w_hh0, g, start=True)

                # ---- finish layer 0: x/bias matmul + activations
                for g in PRE_GROUPS:
                    nc.tensor.matmul(ps_l0[g][:], xa[:],
                                     k9[:, g * NG:(g + 1) * NG],
                                     start=False, stop=True)
                    act_for_group(0, g, ps_l0[g])
                for g in FULL_GROUPS:
                    ps = gpsum.tile([BL, NG], f32, tag="gps")
                    dr_mms(ps, h0T8, w_hh0, g, start=True)
                    nc.tensor.matmul(ps[:], xa[:],
                                     k9[:, g * NG:(g + 1) * NG],
                                     start=False, stop=True)
                    act_for_group(0, g, ps)

                cell_math(0)

                # ---- layer 1 gates: h1 @ W_hh1.T (A) + h0new @ W_ih1.T + b1 (B)
                g1ps = [None] * NGROUPS

                def emit_A(g):
                    ps = gpsum.tile([BL, NG], f32, tag="gps")
                    g1ps[g] = ps
                    dr_mms(ps, h1T8, w_hh1, g, start=True)

                def emit_B(g):
                    ps = g1ps[g]
                    if g in (4, 5):   # tanh(g-gate) columns stay fp16
                        fp16_mms(ps, h0T16, w_ih1g, (g - 4) * NG, start=False)
                    else:             # i/f/o columns: sigmoid saturates, fp8 ok
                        dr_mms(ps, h0T8, w_ih1, g, start=False)
                    nc.tensor.matmul(ps[:], xbias[:],
                                     k9[:, g * NG:(g + 1) * NG],
                                     start=False, stop=True)
                    act_for_group(1, g, ps)

                # A-parts depend only on the previous step's h1; emit a
                # couple ahead of the layer-0 transposes so PE stays busy
                # while the cell-0 chain produces h0_new, and interleave the
                # rest with B in cell-friendly order (i, g, f, o).
                emit_A(0)
                emit_A(4)
                transpose_h(0)
                emit_A(1)
                emit_B(0)
                emit_A(5)
                emit_B(4)
                emit_A(2)
                emit_B(1)
                emit_A(6)
                emit_B(5)
                emit_A(3)
                emit_B(2)
                emit_A(7)
                emit_B(3)
                emit_B(6)
                emit_B(7)

                cell_math(1)

                # pre-emit next step's layer-0 DR matmuls: they only need
                # h0T8 (ready) and keep the PE busy while cell-1 finishes;
                # one group before the transpose covers the cell tail, the
                # rest land after y so the y->xa feedback latency hides.
                if t + 1 < Tsteps:
                    xan = xaug[(t + 1) % 3]
                    nc.sync.dma_start(xan[1:9, :], d_ffy[t + 1, 1:9, :])

                    def l0pre(g):
                        ps = gpsum.tile([BL, NG], f32, tag="gps")
                        ps_l0[g] = ps
                        dr_mms(ps, h0T8, w_hh0, g, start=True)

                    l0pre(PRE_GROUPS[0])
                transpose_h(1)

                # ---- output projection: y^T = W_out @ h1^T + b_out
                # (split in k-halves so the first half streams while the
                # second h1T16 copy lands)
                yp = ypsum.tile([Q, BL], f32, tag="yp")
                for k in range(KT):
                    nc.tensor.matmul(yp[:], w_out[:, k, :], h1T16[:, k, :],
                                     start=(k == 0), stop=(k == KT - 1))
                # feed y[:, 0] straight into next step's xa row via a tiny
                # ACT (keeps the y->xa chain off the DVE queue)
                if t + 1 < Tsteps:
                    nc.scalar.activation(xaug[(t + 1) % 3][0:1, :],
                                         yp[0:1, :], IDENT,
                                         bias=b_out[0:1, 0:1])
                yts = ytp.tile([Q, BL], f32, tag="yts")
                nc.scalar.activation(yts[:], yp[:], IDENT, bias=b_out[:, 0:1])
                nc.sync.dma_start(d_y[t], yts[:])
                if t + 1 < Tsteps:
                    for g in PRE_GROUPS[1:]:
                        l0pre(g)

    _split_multi_waits(nc)
    return nc


def _q8(x, scale):
    y = np.clip(np.asarray(x, np.float32) * scale, -240, 240)
    return y.astype(ml_dtypes.float8_e4m3fn)


def _pack_weights(inputs):
    def t_pack(w):  # W [4H, K] -> W.T [K, 4H] -> [128, K/128, 4H] fp32
        wt = np.ascontiguousarray(np.asarray(w, np.float32).T)
        k = wt.shape[0]
        return np.ascontiguousarray(
            wt.reshape(k // 128, 128, wt.shape[1]).transpose(1, 0, 2))

    whh0t = _q8(t_pack(inputs["W_hh0"]), WS)
    whh1t = _q8(t_pack(inputs["W_hh1"]), WS)
    wih1_f = t_pack(inputs["W_ih1"])
    wih1t = _q8(wih1_f, WS)
    # fp16 copy of the g-gate (cell candidate) columns 2048:3072
    wih1g = np.ascontiguousarray(
        wih1_f[:, :, 2 * H:3 * H] * GS).astype(np.float16)

    k9 = np.zeros((128, G4H), np.float32)
    W_ih0 = np.asarray(inputs["W_ih0"], np.float32)  # [4H, 8]
    k9[0, :] = W_ih0[:, 0]
    k9[1:8, :] = W_ih0[:, 1:8].T
    k9[8, :] = np.asarray(inputs["b_ih0"], np.float32) + np.asarray(
        inputs["b_hh0"], np.float32)
    k9[9, :] = np.asarray(inputs["b_ih1"], np.float32) + np.asarray(
        inputs["b_hh1"], np.float32)
    k9 = (k9 * GS).astype(np.float16)

    woutT = np.asarray(inputs["W_out"], np.float32).T  # [H, 9]
    woutt = np.ascontiguousarray(
        woutT.reshape(KT, 128, Q).transpose(1, 0, 2)).astype(np.float16)
    bout = np.asarray(inputs["b_out"], np.float32).reshape(Q, 1)
    return whh0t, whh1t, wih1t, wih1g, k9, woutt, bout


def kernel(**inputs):
    return _run(inputs, T)


def _run(inputs, Tsteps, trace=False):
    if Tsteps not in _module_cache:
        _module_cache[Tsteps] = _build_module(Tsteps)
    nc = _module_cache[Tsteps]

    whh0t, whh1t, wih1t, wih1g, k9, woutt, bout = _pack_weights(inputs)

    h = np.asarray(inputs["h"], np.float32)     # [2, B, H]
    c = np.asarray(inputs["c"], np.float32)
    ff = np.asarray(inputs["future_features"], np.float32)[:, :Tsteps]  # [B, T, 7]
    y0 = np.asarray(inputs["inp_y"], np.float32)[:, 0, 0]   # [B]

    in_maps = []
    for core in range(NCORES):
        s = slice(core * BL, (core + 1) * BL)

        def h_packT(hl):  # h [BL, H] -> h.T [H, BL] -> [128, KT, BL]
            ht = np.ascontiguousarray(np.asarray(hl, np.float32).T)
            return np.ascontiguousarray(
                ht.reshape(KT, 128, BL).transpose(1, 0, 2))

        h0t_f = h_packT(h[0, s])
        h1t_f = h_packT(h[1, s])

        ffy = np.zeros((Tsteps, 9, BL), np.float32)
        ffy[0, 0, :] = y0[s]
        ffy[:, 1:8, :] = ff[s].transpose(1, 2, 0)  # [T, 7, BL]
        ffy[:, 8, :] = 1.0

        xbias_np = np.zeros((128, 128), np.float16)
        xbias_np[9, :] = 1.0
        in_maps.append({
            "whh0t": whh0t,
            "whh1t": whh1t,
            "wih1t": wih1t,
            "wih1g": wih1g,
            "k9pack": k9,
            "woutt": woutt,
            "bout": bout,
            "xbias": xbias_np,
            "h0t8": _q8(h0t_f, HS),
            "h1t8": _q8(h1t_f, HS),
            "h0t16": h0t_f.astype(np.float16),
            "h1t16": h1t_f.astype(np.float16),
            "cinit": np.ascontiguousarray(c[:, s, :]),
            "ffy": ffy.astype(np.float16),
        })

    res = run_bass_kernel_spmd(nc, in_maps, core_ids=list(range(NCORES)),
                               trace=trace)
    _run.last_result = res

    out = np.empty((B, Tsteps, Q), np.float32)
    for core in range(NCORES):
        s = slice(core * BL, (core + 1) * BL)
        out[s] = res.results[core]["yout"].transpose(2, 0, 1)  # [BL, T, 9]
    return out.reshape(B, Tsteps, M, Q)
